# revision 6
# baseline (speedup 1.0000x reference)
"""Multi-head causal self-attention (d_model=1024, 16 heads, seq 2048, batch 4)
as a Bass/Tile kernel for 8 Trainium2 NeuronCores.

Sharding: core c = (batch b = c//2, head-group g = c%2); each group = 8 heads
(512 features), processed as 4 head-PAIRS. Per core:
  - QKV projection for its batch, its group's slice of w_qkv
  - causal attention for its 8 heads (S^T layout, softmax without
    max-subtraction: logits ~ N(0,1), exp is safe in fp16)
  - partial output projection y_part = attn_g @ w_out[g*512:(g+1)*512, :]
Host: y[b] = y_part[2b] + y_part[2b+1] + b_out.

PE-array packing: the two heads of a pair occupy SBUF partitions 0-63 /
64-127, so their K=64 score matmuls run CONCURRENTLY in the top/bottom
row-groups of the 128x128 array (row tiling), and their 64-wide P@V
matmuls run concurrently in the left/right column-groups (col tiling,
tile_position=(0,0)/(0,64)) accumulating into one PSUM bank.  Softmax
denominators are accumulated on DVE from the exp'd tiles (fp16) and
collapsed across partitions with a single ones-vector matmul.

All matmul operands fp16 (PE streams 1 cycle/row vs 4 for fp32);
accumulation fp32 in PSUM.  Host-side input blobs are laid out as the
exact SBUF image ([128, X], >=4KB contiguous per partition row) so input
DMA runs at full HBM rate with ~12 descriptors.
"""
import sys
import types

if "/opt/trn_rl_repo" not in sys.path:
    sys.path.insert(0, "/opt/trn_rl_repo")

import numpy as np

import concourse.bacc as bacc
import concourse.bass as bass
import concourse.mybir as mybir
import concourse.tile as tile
from concourse.bass_utils import run_bass_kernel_spmd
from concourse.masks import make_upper_triangular

D_MODEL = 1024
N_SEQ = 2048
N_HEADS_G = 8          # heads per core (group)
D_HEAD = 64
F_G = N_HEADS_G * D_HEAD   # 512 features per group
N_CORES = 8
N_PAIRS = 4            # head pairs per core
NCHUNK = 4             # 512-wide seq chunks

FP16 = mybir.dt.float16
FP32 = mybir.dt.float32

KB = D_MODEL // 128    # 8 k-blocks


def _build_program():
    nc = bacc.Bacc("TRN2", target_bir_lowering=False, debug=False,
                   num_devices=N_CORES)

    # DRAM blobs are exact SBUF images ([128 partitions, X cols]).
    xT = nc.dram_tensor("xT", [128, NCHUNK * KB * 512], FP16, kind="ExternalInput")
    wqk = nc.dram_tensor("wqk", [128, N_PAIRS * 2048], FP16, kind="ExternalInput")
    wv = nc.dram_tensor("wv", [128, KB * 512], FP16, kind="ExternalInput")
    wout = nc.dram_tensor("wout", [128, 4 * D_MODEL], FP16, kind="ExternalInput")
    bqk = nc.dram_tensor("bqk", [128, 8], FP32, kind="ExternalInput")
    bv = nc.dram_tensor("bv", [128, F_G], FP32, kind="ExternalInput")
    y = nc.dram_tensor("y", [N_SEQ, D_MODEL], FP16, kind="ExternalOutput")

    with tile.TileContext(nc) as tc:
        _emit(nc, tc, xT, wqk, wv, wout, bqk, bv, y)
    nc.compile()
    return nc


def _emit(nc, tc, xT, wqk, wv, wout, bqk, bv, y):
    import contextlib
    ctx = contextlib.ExitStack()
    with ctx:
        persist = ctx.enter_context(tc.tile_pool(name="persist", bufs=1))
        pt_p = ctx.enter_context(tc.tile_pool(name="pt", bufs=6))
        sm_p = ctx.enter_context(tc.tile_pool(name="sm", bufs=2))
        rc_p = ctx.enter_context(tc.tile_pool(name="rc", bufs=3))
        yb_p = ctx.enter_context(tc.tile_pool(name="yb", bufs=2))
        # PSUM: "s" [128,1024]x2 = 4 banks; "acc" [128,512]x2 = 2 banks;
        # "mm" [128,512]x2 = 2 banks (qk/v/proj matmuls + sums share it).
        s_ps = ctx.enter_context(tc.tile_pool(name="sps", bufs=2, space="PSUM"))
        acc_ps = ctx.enter_context(tc.tile_pool(name="accps", bufs=2, space="PSUM"))
        mm_ps = ctx.enter_context(tc.tile_pool(name="mmps", bufs=2, space="PSUM"))

        # ---- persistent SBUF tensors ----
        xT_sb = persist.tile([128, NCHUNK * KB * 512], FP16, tag="xT")
        wqk_sb = persist.tile([128, N_PAIRS * 2048], FP16, tag="wqk")
        wv_sb = persist.tile([128, KB * 512], FP16, tag="wv")
        wout_sb = persist.tile([128, 4 * D_MODEL], FP16, tag="wout")
        bqk_sb = persist.tile([128, 8], FP32, tag="bqk")
        bv_sb = persist.tile([128, F_G], FP32, tag="bv")
        tri_sb = persist.tile([128, 128], FP16, tag="tri")
        ones_sb = persist.tile([128, 1], FP16, tag="ones")
        qt_sb = [persist.tile([128, N_SEQ], FP16, tag=f"qt{p}", name=f"qt{p}")
                 for p in range(N_PAIRS)]
        kt_sb = [persist.tile([128, N_SEQ], FP16, tag=f"kt{p}", name=f"kt{p}")
                 for p in range(N_PAIRS)]
        v_sb = [persist.tile([128, F_G], FP16, tag=f"v{j}", name=f"v{j}")
                for j in range(16)]
        attnT_sb = [persist.tile([128, N_SEQ], FP16, tag=f"attnT{p}", name=f"attnT{p}")
                    for p in range(N_PAIRS)]

        # ---- input DMAs, priority order (first matmul needs wqk pair0 + xT c0)
        def load_xt(c):
            nc.sync.dma_start(out=xT_sb[:, c * 4096:(c + 1) * 4096],
                              in_=xT.ap()[:, c * 4096:(c + 1) * 4096])

        def load_wqk(p):
            nc.sync.dma_start(out=wqk_sb[:, p * 2048:(p + 1) * 2048],
                              in_=wqk.ap()[:, p * 2048:(p + 1) * 2048])

        load_wqk(0)
        load_xt(0)
        nc.sync.dma_start(out=bqk_sb[:], in_=bqk.ap())
        nc.sync.dma_start(out=bv_sb[:], in_=bv.ap())
        nc.sync.dma_start(out=wv_sb[:], in_=wv.ap())
        load_xt(1)
        load_wqk(1)
        nc.sync.dma_start(out=wout_sb[:], in_=wout.ap())
        load_xt(2)
        load_wqk(2)
        load_xt(3)
        load_wqk(3)

        # upper-triangular (incl diag) ones mask: tri[j, i] = 1 iff i >= j
        make_upper_triangular(nc, tri_sb[:], val=1.0, diag=True)
        nc.vector.memset(ones_sb[:], 1.0)

        # ---- QKV projection pieces ----
        def emit_qk_group(p, qk, c):
            # q (qk=0) or k (qk=1) features of head pair p, seq chunk c
            ps = mm_ps.tile([128, 512], FP32, tag="mm", name="qkps")
            for kb in range(KB):
                nc.tensor.matmul(
                    ps[:],
                    wqk_sb[:, p * 2048 + qk * 1024 + kb * 128:
                           p * 2048 + qk * 1024 + (kb + 1) * 128],
                    xT_sb[:, c * 4096 + kb * 512:c * 4096 + (kb + 1) * 512],
                    start=(kb == 0), stop=(kb == KB - 1),
                )
            dest = kt_sb[p] if qk else qt_sb[p]
            nc.vector.tensor_scalar_add(
                dest[:, c * 512:(c + 1) * 512], ps[:],
                bqk_sb[:, qk * 4 + p:qk * 4 + p + 1])

        def emit_v_block(ib):
            c = ib // 4
            ps = mm_ps.tile([128, 512], FP32, tag="mm", name="vps")
            for kb in range(KB):
                nc.tensor.matmul(
                    ps[:],
                    xT_sb[:, c * 4096 + kb * 512 + (ib % 4) * 128:
                          c * 4096 + kb * 512 + (ib % 4 + 1) * 128],
                    wv_sb[:, kb * 512:(kb + 1) * 512],
                    start=(kb == 0), stop=(kb == KB - 1),
                )
            nc.vector.tensor_add(v_sb[ib][:], ps[:], bv_sb[:])

        # ---- causal attention for one head pair, one 512-i chunk ----
        # S^T layout: s[j_local, i] per j-block; heads A/B run concurrently
        # (A: array rows 0-63, B: rows 64-127 -- from the SBUF partition
        # ranges of qt/kt).  P@V col-tiled: A -> psum partitions 0-63, B ->
        # 64-127 of one accumulator bank.  Row 'sums' accumulated on DVE.
        def emit_attn_pair(p, c, fillers):
            nm = 2 * c + 2  # m-tiles of 2 j-blocks
            accAB = acc_ps.tile([128, 512], FP32, tag="acc", name="accAB")
            accsum = sm_p.tile([128, 1024], FP16, tag="accsum", name="accsum")
            cols = slice(c * 512, (c + 1) * 512)
            for m in range(nm):
                if fillers:
                    fillers.pop(0)()
                if m == nm - 1:
                    while fillers:   # flush unconsumed filler work
                        fillers.pop(0)()
                jb0, jb1 = 2 * m, 2 * m + 1
                t0, t1 = jb0 - 4 * c, jb1 - 4 * c
                off0, off1 = max(0, t0) * 128, max(0, t1) * 128
                s_t = [None, None]
                # scores: interleave heads so the two K=64 matmuls overlap.
                # Half 1 also starts at off0 (not off1) so the exp below
                # reads fully-initialized PSUM; cols [off0,off1) of half 1
                # are above-diagonal junk that no consumer reads.
                for half, jb in enumerate((jb0, jb1)):
                    for hl in range(2):
                        if half == 0:
                            s_t[hl] = s_ps.tile([128, 1024], FP32, tag="s",
                                                name="spsAB")
                        nc.tensor.matmul(
                            s_t[hl][:, half * 512 + off0:(half + 1) * 512],
                            kt_sb[p][hl * 64:hl * 64 + 64,
                                     jb * 128:(jb + 1) * 128],
                            qt_sb[p][hl * 64:hl * 64 + 64,
                                     c * 512 + off0:(c + 1) * 512],
                            start=True, stop=True,
                        )
                pt_t = []
                for hl in range(2):
                    pt = pt_p.tile([128, 1024], FP16, tag="pt", name="pt")
                    pt_t.append(pt)
                    # gap cols [512+off0, 512+off1) get exp(stale) - never read
                    nc.scalar.activation(pt[:, off0:], s_t[hl][:, off0:],
                                         mybir.ActivationFunctionType.Exp)
                    # mask diagonal sub-blocks
                    if 0 <= t0 and t1 <= 3:
                        blk = bass.AP(tensor=pt.tensor,
                                      offset=pt.offset + t0 * 128,
                                      ap=[list(pt.ap[0]), [640, 2], [1, 128]])
                        tri2 = bass.AP(tensor=tri_sb.tensor,
                                       offset=tri_sb.offset,
                                       ap=[list(tri_sb.ap[0]), [0, 2], [1, 128]])
                        nc.vector.tensor_mul(blk, blk, tri2)
                    else:
                        for half, t in ((0, t0), (1, t1)):
                            if 0 <= t <= 3:
                                sl = slice(half * 512 + t * 128,
                                           half * 512 + (t + 1) * 128)
                                nc.vector.tensor_mul(pt[:, sl], pt[:, sl],
                                                     tri_sb[:])
                    # denominator partials: accsum[:, hl*512+i] += pt halves
                    # (half 1 from off1 -- its [off0,off1) region is junk)
                    asl = accsum[:, hl * 512:hl * 512 + 512]
                    if m == 0:
                        nc.vector.tensor_copy(asl[:], pt[:, 0:512])
                        nc.vector.tensor_add(asl[:, off1:], asl[:, off1:],
                                             pt[:, 512 + off1:1024])
                    else:
                        for half, off in ((0, off0), (1, off1)):
                            nc.vector.tensor_add(
                                asl[:, off:], asl[:, off:],
                                pt[:, half * 512 + off:(half + 1) * 512])
                # P@V: the two heads' 64-wide matmuls run col-concurrent
                for half, (jb, off) in enumerate(((jb0, off0), (jb1, off1))):
                    for hl in range(2):
                        nc.tensor.matmul(
                            accAB[hl * 64:(hl + 1) * 64, off:512],
                            v_sb[jb][:, (2 * p + hl) * 64:(2 * p + hl + 1) * 64],
                            pt_t[hl][:, half * 512 + off:(half + 1) * 512],
                            start=(m == 0 and half == 0),
                            stop=(m == nm - 1 and half == 1),
                            tile_position=(0, 64 * hl),
                        )
            # denominators: ones^T @ accsum -> [1,512] per head, evict, recip
            sums_sb = rc_p.tile([1, 1024], FP32, tag="sums", name="sums")
            for hl in range(2):
                sums_ps = mm_ps.tile([1, 512], FP32, tag="mm", name="sumsps")
                nc.tensor.matmul(sums_ps[:], ones_sb[:],
                                 accsum[:, hl * 512:(hl + 1) * 512],
                                 start=True, stop=True)
                nc.vector.tensor_copy(sums_sb[:, hl * 512:(hl + 1) * 512],
                                      sums_ps[:])
            s48 = rc_p.tile([128, 8], FP32, tag="s48", name="s48")
            nc.sync.dma_start(out=s48[:], in_=sums_sb[:])
            r48 = rc_p.tile([128, 8], FP32, tag="r48", name="r48")
            nc.vector.reciprocal(r48[:], s48[:])
            r48h = rc_p.tile([128, 8], FP16, tag="r48h", name="r48h")
            nc.vector.tensor_copy(r48h[:], r48[:])
            rr = rc_p.tile([1, 1024], FP16, tag="rr", name="rr")
            nc.sync.dma_start(out=rr[:], in_=r48h[:])
            rep = rc_p.tile([128, 1024], FP16, tag="rep", name="rep")
            nc.gpsimd.partition_broadcast(rep[:], rr[:])
            # evict P@V accumulator and normalize (partition-aligned per head)
            ou = rc_p.tile([128, 512], FP16, tag="ou", name="ou")
            nc.vector.tensor_copy(ou[:], accAB[:])
            nc.vector.tensor_mul(attnT_sb[p][0:64, cols], ou[0:64, :],
                                 rep[0:64, 0:512])
            nc.vector.tensor_mul(attnT_sb[p][64:128, cols], ou[64:128, :],
                                 rep[64:128, 512:1024])

        # ---- output projection ----
        y_sb = {}

        def emit_proj(ib, ec):
            ps = mm_ps.tile([128, 512], FP32, tag="mm", name="projps")
            for fbp in range(N_PAIRS):
                nc.tensor.matmul(
                    ps[:],
                    attnT_sb[fbp][:, ib * 128:(ib + 1) * 128],
                    wout_sb[:, fbp * 1024 + ec * 512:fbp * 1024 + (ec + 1) * 512],
                    start=(fbp == 0), stop=(fbp == N_PAIRS - 1),
                )
            if ib not in y_sb:
                y_sb[ib] = yb_p.tile([128, D_MODEL], FP16, tag="ysb", name="ysb")
            nc.vector.tensor_copy(y_sb[ib][:, ec * 512:(ec + 1) * 512], ps[:])
            if ec == 1:
                nc.sync.dma_start(
                    out=y.ap()[ib * 128:(ib + 1) * 128, :], in_=y_sb.pop(ib)[:])

        # ---- schedule ----
        emit_qk_group(0, 0, 0)
        emit_qk_group(0, 1, 0)
        for ib in range(4):
            emit_v_block(ib)

        pending_proj = []

        for c in range(NCHUNK):
            for p in range(N_PAIRS):
                fillers = []
                # next attention block's q/k projections
                np_, nc_ = (p + 1, c) if p < N_PAIRS - 1 else (0, c + 1)
                if nc_ < NCHUNK:
                    fillers.append(lambda np_=np_, nc_=nc_: emit_qk_group(np_, 0, nc_))
                    fillers.append(lambda np_=np_, nc_=nc_: emit_qk_group(np_, 1, nc_))
                # V blocks needed soon
                if c == 0:
                    vb0 = 4 * (p + 1)
                    for ib in range(vb0, min(vb0 + 4, 16)):
                        fillers.append(lambda ib=ib: emit_v_block(ib))
                # trickle output projections for completed chunks
                def pump_proj():
                    for _ in range(2):
                        if pending_proj:
                            emit_proj(*pending_proj.pop(0))
                fillers += [pump_proj] * max(0, (2 * c + 2) - len(fillers))
                emit_attn_pair(p, c, fillers)
                if p == N_PAIRS - 1:
                    pending_proj += [(ib, ec) for ib in range(4 * c, 4 * c + 4)
                                     for ec in range(2)]
        while pending_proj:
            emit_proj(*pending_proj.pop(0))


_NC_CACHE = None


def _get_nc():
    global _NC_CACHE
    if _NC_CACHE is None:
        _NC_CACHE = _build_program()
    return _NC_CACHE


def _make_in_maps(x, w_qkv, b_qkv, w_out):
    scale = D_HEAD ** -0.5
    in_maps = []
    for core in range(N_CORES):
        b, g = core // 2, core % 2
        f0 = g * F_G
        # xT blob: [p, c*4096 + k*512 + f] = x[b, c*512+f, k*128+p]
        xt = np.ascontiguousarray(x[b].T).astype(np.float16)        # [1024, 2048]
        xt_blob = xt.reshape(KB, 128, NCHUNK, 512).transpose(1, 2, 0, 3) \
                    .reshape(128, NCHUNK * KB * 512)
        # wqk blob: [p, pair*2048 + qk*1024 + kb*128 + f]
        wq = (w_qkv[:, f0:f0 + F_G] * scale).astype(np.float16)      # [1024, 512]
        wk = w_qkv[:, D_MODEL + f0:D_MODEL + f0 + F_G].astype(np.float16)
        wqk_s = np.stack([wq, wk], axis=1)                           # [1024, 2, 512]
        wqk_blob = wqk_s.reshape(KB, 128, 2, N_PAIRS, 128) \
                        .transpose(1, 3, 2, 0, 4).reshape(128, N_PAIRS * 2048)
        # wv blob: [p, kb*512 + f]
        wv_ = w_qkv[:, 2 * D_MODEL + f0:2 * D_MODEL + f0 + F_G].astype(np.float16)
        wv_blob = wv_.reshape(KB, 128, F_G).transpose(1, 0, 2).reshape(128, KB * 512)
        # wout blob: [p, fb*1024 + e] = w_out[f0 + fb*128 + p, e]
        wo = w_out[f0:f0 + F_G, :].astype(np.float16)                # [512, 1024]
        wout_blob = wo.reshape(4, 128, D_MODEL).transpose(1, 0, 2) \
                      .reshape(128, 4 * D_MODEL)
        # bqk: [p, qk*4 + pair]
        bq = (b_qkv[f0:f0 + F_G] * scale).astype(np.float32).reshape(N_PAIRS, 128)
        bk = b_qkv[D_MODEL + f0:D_MODEL + f0 + F_G].astype(np.float32) \
            .reshape(N_PAIRS, 128)
        bqk_blob = np.concatenate([bq, bk], axis=0).T                # [128, 8]
        bv_ = b_qkv[2 * D_MODEL + f0:2 * D_MODEL + f0 + F_G].astype(np.float32)
        in_maps.append({
            "xT": np.ascontiguousarray(xt_blob),
            "wqk": np.ascontiguousarray(wqk_blob),
            "wv": np.ascontiguousarray(wv_blob),
            "wout": np.ascontiguousarray(wout_blob),
            "bqk": np.ascontiguousarray(bqk_blob),
            "bv": np.broadcast_to(bv_, (128, F_G)).copy(),
        })
    return in_maps


def _register_ntff_hook():
    try:
        import antenv.axon_hooks  # noqa: F401
        return
    except ImportError:
        pass
    try:
        from trn_agent_boot.trn_boot import _ntff_profile_via_ctypes
        hook = _ntff_profile_via_ctypes("/opt/axon/libaxon_pjrt.so")
        mod = types.ModuleType("antenv.axon_hooks")
        mod.get_axon_ntff_profile_hook = lambda: hook
        sys.modules["antenv.axon_hooks"] = mod
    except Exception:
        pass


def run(x, w_qkv, b_qkv, w_out, b_out, trace=False, tmpdir=None):
    x = np.asarray(x, dtype=np.float32)
    w_qkv = np.asarray(w_qkv, dtype=np.float32)
    b_qkv = np.asarray(b_qkv, dtype=np.float32)
    w_out = np.asarray(w_out, dtype=np.float32)
    b_out = np.asarray(b_out, dtype=np.float32)

    nc = _get_nc()
    in_maps = _make_in_maps(x, w_qkv, b_qkv, w_out)
    if trace:
        _register_ntff_hook()
    res = run_bass_kernel_spmd(nc, in_maps, core_ids=list(range(N_CORES)),
                               trace=trace, tmpdir=tmpdir)
    bsz = x.shape[0]
    out = np.empty((bsz, N_SEQ, D_MODEL), np.float32)
    for b in range(bsz):
        out[b] = (res.results[2 * b]["y"].astype(np.float32)
                  + res.results[2 * b + 1]["y"].astype(np.float32)
                  + b_out[None, :])
    return out, res


def kernel(x, w_qkv, b_qkv, w_out, b_out):
    out, _ = run(x, w_qkv, b_qkv, w_out, b_out, trace=False)
    return out


# revision 21
# speedup vs baseline: 1.2234x; 1.2234x over previous
"""Multi-head causal self-attention (d_model=1024, 16 heads, seq 2048, batch 4)
as a Bass/Tile kernel for 8 Trainium2 NeuronCores.

Sharding: core c = (batch b = c//2, head-group g = c%2); each group = 8 heads
(512 features), processed as 4 head-PAIRS. Per core:
  - QKV projection for its batch, its group's slice of w_qkv
  - causal attention for its 8 heads (S^T layout, softmax without
    max-subtraction: logits ~ N(0,1), exp is safe in fp16)
  - partial output projection y_part = attn_g @ w_out[g*512:(g+1)*512, :]
Host: y[b] = y_part[2b] + y_part[2b+1] + b_out.

PE-array packing: the two heads of a pair occupy SBUF partitions 0-63 /
64-127, so their K=64 score matmuls run CONCURRENTLY in the top/bottom
row-groups of the 128x128 array (row tiling), and their 64-wide P@V
matmuls run concurrently in the left/right column-groups (col tiling,
tile_position=(0,0)/(0,64)) accumulating into one PSUM bank.  Softmax
denominators are accumulated on DVE from the exp'd tiles (fp16) and
collapsed across partitions with a single ones-vector matmul.

All matmul operands fp16 (PE streams 1 cycle/row vs 4 for fp32);
accumulation fp32 in PSUM.  Host-side input blobs are laid out as the
exact SBUF image ([128, X], >=4KB contiguous per partition row) so input
DMA runs at full HBM rate with ~12 descriptors.
"""
import sys
import types

if "/opt/trn_rl_repo" not in sys.path:
    sys.path.insert(0, "/opt/trn_rl_repo")

import numpy as np

import concourse.bacc as bacc
import concourse.bass as bass
import concourse.mybir as mybir
import concourse.tile as tile
from concourse.bass_utils import run_bass_kernel_spmd
from concourse.masks import make_upper_triangular

D_MODEL = 1024
N_SEQ = 2048
N_HEADS_G = 8          # heads per core (group)
D_HEAD = 64
F_G = N_HEADS_G * D_HEAD   # 512 features per group
N_CORES = 8
N_PAIRS = 4            # head pairs per core
NCHUNK = 4             # 512-wide seq chunks

FP16 = mybir.dt.float16
FP32 = mybir.dt.float32

KB = D_MODEL // 128    # 8 k-blocks


def _build_program():
    nc = bacc.Bacc("TRN2", target_bir_lowering=False, debug=False,
                   num_devices=N_CORES)

    # DRAM blobs are exact SBUF images ([128 partitions, X cols]).
    xT = nc.dram_tensor("xT", [128, NCHUNK * KB * 512], FP16, kind="ExternalInput")
    wqk = nc.dram_tensor("wqk", [128, N_PAIRS * 2048], FP16, kind="ExternalInput")
    wv = nc.dram_tensor("wv", [128, KB * 512], FP16, kind="ExternalInput")
    wout = nc.dram_tensor("wout", [128, 4 * D_MODEL], FP16, kind="ExternalInput")
    bqk = nc.dram_tensor("bqk", [128, 8], FP32, kind="ExternalInput")
    bv = nc.dram_tensor("bv", [128, F_G], FP32, kind="ExternalInput")
    y = nc.dram_tensor("y", [N_SEQ, D_MODEL], FP16, kind="ExternalOutput")
    dbg = {}
    if DEBUG_OUTPUTS:
        dbg["qt"] = nc.dram_tensor("qt_dbg", [4, 128, N_SEQ], FP16,
                                   kind="ExternalOutput")
        dbg["kt"] = nc.dram_tensor("kt_dbg", [4, 128, N_SEQ], FP16,
                                   kind="ExternalOutput")
        dbg["v"] = nc.dram_tensor("v_dbg", [16, 128, N_HEADS_G, D_HEAD + 1],
                                  FP16, kind="ExternalOutput")
        dbg["attnT"] = nc.dram_tensor("attnT_dbg", [4, 128, N_SEQ], FP16,
                                      kind="ExternalOutput")

    with tile.TileContext(nc) as tc:
        _emit(nc, tc, xT, wqk, wv, wout, bqk, bv, y, dbg)
    nc.compile()
    return nc


DEBUG_OUTPUTS = False


def _emit(nc, tc, xT, wqk, wv, wout, bqk, bv, y, dbg=None):
    import contextlib
    ctx = contextlib.ExitStack()
    with ctx:
        persist = ctx.enter_context(tc.tile_pool(name="persist", bufs=1))
        pt_p = ctx.enter_context(tc.tile_pool(name="pt", bufs=6))
        rc_p = ctx.enter_context(tc.tile_pool(name="rc", bufs=3))
        yb_p = ctx.enter_context(tc.tile_pool(name="yb", bufs=2))
        # PSUM: "s" [128,1024]x2 = 4 banks; "acc" [128,512]x2 = 2 banks;
        # "mm" [128,512]x2 = 2 banks (qk/v/proj matmuls share it).
        s_ps = ctx.enter_context(tc.tile_pool(name="sps", bufs=2, space="PSUM"))
        acc_ps = ctx.enter_context(tc.tile_pool(name="accps", bufs=2, space="PSUM"))
        mm_ps = ctx.enter_context(tc.tile_pool(name="mmps", bufs=2, space="PSUM"))

        # ---- persistent SBUF tensors ----
        xT_sb = persist.tile([128, NCHUNK * KB * 512], FP16, tag="xT")
        wqk_sb = persist.tile([128, N_PAIRS * 2048], FP16, tag="wqk")
        wv_sb = persist.tile([128, KB * 512], FP16, tag="wv")
        wout_sb = persist.tile([128, 4 * D_MODEL], FP16, tag="wout")
        bqk_sb = persist.tile([128, 8], FP32, tag="bqk")
        bv_sb = persist.tile([128, F_G], FP32, tag="bv")
        tri_sb = persist.tile([128, 128], FP16, tag="tri")
        qt_sb = [persist.tile([128, N_SEQ], FP16, tag=f"qt{p}", name=f"qt{p}")
                 for p in range(N_PAIRS)]
        kt_sb = [persist.tile([128, N_SEQ], FP16, tag=f"kt{p}", name=f"kt{p}")
                 for p in range(N_PAIRS)]
        v_sb = [persist.tile([128, N_HEADS_G, D_HEAD + 1], FP16, tag=f"v{j}",
                             name=f"v{j}")
                for j in range(16)]
        attnT_sb = [persist.tile([128, N_SEQ], FP16, tag=f"attnT{p}", name=f"attnT{p}")
                    for p in range(N_PAIRS)]

        # ---- input DMAs, priority order (first matmul needs wqk pair0 + xT c0)
        def load_xt(c, half=None):
            sl = slice(c * 4096, (c + 1) * 4096) if half is None else \
                slice(c * 4096 + half * 2048, c * 4096 + (half + 1) * 2048)
            nc.sync.dma_start(out=xT_sb[:, sl], in_=xT.ap()[:, sl])

        def load_wqk(p):
            nc.sync.dma_start(out=wqk_sb[:, p * 2048:(p + 1) * 2048],
                              in_=wqk.ap()[:, p * 2048:(p + 1) * 2048])

        load_wqk(0)
        load_xt(0, 0)
        load_xt(0, 1)
        nc.sync.dma_start(out=bqk_sb[:], in_=bqk.ap())
        nc.sync.dma_start(out=bv_sb[:], in_=bv.ap())
        nc.sync.dma_start(out=wv_sb[:], in_=wv.ap())
        load_xt(1)
        load_wqk(1)
        nc.sync.dma_start(out=wout_sb[:], in_=wout.ap())
        load_xt(2)
        load_wqk(2)
        load_xt(3)
        load_wqk(3)

        # upper-triangular (incl diag) ones mask: tri[j, i] = 1 iff i >= j
        make_upper_triangular(nc, tri_sb[:], val=1.0, diag=True)
        # ones column for the fused row-sum in P@V
        for j in range(16):
            nc.vector.memset(v_sb[j][:, :, D_HEAD:D_HEAD + 1], 1.0)

        # ---- QKV projection pieces ----
        def emit_qk_group(p, qk, c):
            # q (qk=0) or k (qk=1) features of head pair p, seq chunk c
            ps = mm_ps.tile([128, 512], FP32, tag="mm", name="qkps")
            for kb in range(KB):
                nc.tensor.matmul(
                    ps[:],
                    wqk_sb[:, p * 2048 + qk * 1024 + kb * 128:
                           p * 2048 + qk * 1024 + (kb + 1) * 128],
                    xT_sb[:, c * 4096 + kb * 512:c * 4096 + (kb + 1) * 512],
                    start=(kb == 0), stop=(kb == KB - 1),
                )
            dest = kt_sb[p] if qk else qt_sb[p]
            nc.vector.tensor_scalar_add(
                dest[:, c * 512:(c + 1) * 512], ps[:],
                bqk_sb[:, qk * 4 + p:qk * 4 + p + 1])

        def emit_v_block(ib):
            c = ib // 4
            ps = mm_ps.tile([128, 512], FP32, tag="mm", name="vps")
            for kb in range(KB):
                nc.tensor.matmul(
                    ps[:],
                    xT_sb[:, c * 4096 + kb * 512 + (ib % 4) * 128:
                          c * 4096 + kb * 512 + (ib % 4 + 1) * 128],
                    wv_sb[:, kb * 512:(kb + 1) * 512],
                    start=(kb == 0), stop=(kb == KB - 1),
                )
            nc.vector.tensor_add(
                v_sb[ib][:, :, 0:D_HEAD],
                ps[:].rearrange("p (h d) -> p h d", h=N_HEADS_G),
                bv_sb[:].rearrange("p (h d) -> p h d", h=N_HEADS_G),
            )

        # ---- causal attention for one head pair, one 512-i chunk ----
        # S^T layout, one PSUM tile [128,1024] per j-block holding the score
        # halves of BOTH heads ([A | B]); the two K=64 score matmuls target
        # array row-groups 0-63 / 64-127 and run concurrently (row tiling).
        # exp is one tight 3D-AP ACTIVATE over both halves.  P@V is the
        # 65-wide fused-sum form (row 64 of the accumulator = sum of exp).
        def emit_attn_pair(p, c, fillers):
            njb = 4 * c + 4
            acc = [acc_ps.tile([128, 512], FP32, tag="acc", name=f"acch{hl}")
                   for hl in range(2)]
            cols = slice(c * 512, (c + 1) * 512)
            pend_pv = []
            for jb in range(njb):
                if fillers:
                    fillers.pop(0)()
                if jb == njb - 1:
                    while fillers:   # flush unconsumed filler work
                        fillers.pop(0)()
                t = jb - 4 * c
                off = max(0, t) * 128
                w = 512 - off
                s_t = s_ps.tile([128, 1024], FP32, tag="s", name="sAB")
                for hl in range(2):
                    nc.tensor.matmul(
                        s_t[:, hl * 512 + off:(hl + 1) * 512],
                        kt_sb[p][hl * 64:hl * 64 + 64, jb * 128:(jb + 1) * 128],
                        qt_sb[p][hl * 64:hl * 64 + 64,
                                 c * 512 + off:(c + 1) * 512],
                        start=True, stop=True,
                    )
                pt = pt_p.tile([128, 1024], FP16, tag="pt", name="pt")
                src = bass.AP(tensor=s_t.tensor, offset=s_t.offset + off,
                              ap=[list(s_t.ap[0]), [512, 2], [1, w]])
                dst = bass.AP(tensor=pt.tensor, offset=pt.offset + off,
                              ap=[list(pt.ap[0]), [512, 2], [1, w]])
                nc.scalar.activation(dst, src,
                                     mybir.ActivationFunctionType.Exp)
                if 0 <= t <= 3:
                    blk = bass.AP(tensor=pt.tensor, offset=pt.offset + t * 128,
                                  ap=[list(pt.ap[0]), [512, 2], [1, 128]])
                    tri2 = bass.AP(tensor=tri_sb.tensor, offset=tri_sb.offset,
                                   ap=[list(tri_sb.ap[0]), [0, 2], [1, 128]])
                    nc.vector.tensor_mul(blk, blk, tri2)
                # software-pipeline P@V one j-block behind the scores chain
                pend_pv.append((jb, off, pt))
                if len(pend_pv) > 1:
                    emit_pv(p, c, acc, njb, *pend_pv.pop(0))
            while pend_pv:
                emit_pv(p, c, acc, njb, *pend_pv.pop(0))
            # evict accumulators; row 64 = sum(exp); all-fp16 recip chain
            ou = [rc_p.tile([D_HEAD + 1, 512], FP16, tag=f"ou{hl}",
                            name=f"ou{hl}") for hl in range(2)]
            s48 = rc_p.tile([128, 8], FP16, tag="s48", name="s48")
            for hl in range(2):
                nc.vector.tensor_copy(ou[hl][:], acc[hl][0:D_HEAD + 1, :])
                nc.sync.dma_start(out=s48[:, 4 * hl:4 * hl + 4],
                                  in_=ou[hl][D_HEAD:D_HEAD + 1, :])
            r48 = rc_p.tile([128, 8], FP16, tag="r48", name="r48")
            with nc.allow_low_precision(reason="1/softmax-denominator in fp16 "
                                        "is ~0.05% rel err, well inside gate"):
                nc.vector.reciprocal(r48[:], s48[:])
            rr = rc_p.tile([1, 1024], FP16, tag="rr", name="rr")
            # per-head halves: [1,512] <- [128,4] keeps i = 4p+c identity
            nc.sync.dma_start(out=rr[:, 0:512], in_=r48[:, 0:4])
            nc.sync.dma_start(out=rr[:, 512:1024], in_=r48[:, 4:8])
            rep = rc_p.tile([128, 1024], FP16, tag="rep", name="rep")
            nc.gpsimd.partition_broadcast(rep[:], rr[:])
            nc.vector.tensor_mul(attnT_sb[p][0:64, cols], ou[0][0:D_HEAD, :],
                                 rep[0:64, 0:512])
            tmp = rc_p.tile([64, 512], FP16, tag="tmpB", name="tmpB")
            nc.vector.tensor_mul(tmp[:], ou[1][0:D_HEAD, :],
                                 rep[0:64, 512:1024])
            nc.sync.dma_start(out=attnT_sb[p][64:128, cols], in_=tmp[:])

        def emit_pv(p, c, acc, njb, jb, off, pt):
            for hl in range(2):
                nc.tensor.matmul(
                    acc[hl][0:D_HEAD + 1, off:512],
                    v_sb[jb][:, 2 * p + hl, :],
                    pt[:, hl * 512 + off:(hl + 1) * 512],
                    start=(jb == 0), stop=(jb == njb - 1),
                )

        # ---- output projection ----
        y_sb = {}

        def emit_proj(ib, ec):
            ps = mm_ps.tile([128, 512], FP32, tag="mm", name="projps")
            for fbp in range(N_PAIRS):
                nc.tensor.matmul(
                    ps[:],
                    attnT_sb[fbp][:, ib * 128:(ib + 1) * 128],
                    wout_sb[:, fbp * 1024 + ec * 512:fbp * 1024 + (ec + 1) * 512],
                    start=(fbp == 0), stop=(fbp == N_PAIRS - 1),
                )
            if ib not in y_sb:
                y_sb[ib] = yb_p.tile([128, D_MODEL], FP16, tag="ysb", name="ysb")
            nc.vector.tensor_copy(y_sb[ib][:, ec * 512:(ec + 1) * 512], ps[:])
            if ec == 1:
                nc.sync.dma_start(
                    out=y.ap()[ib * 128:(ib + 1) * 128, :], in_=y_sb.pop(ib)[:])

        # ---- schedule ----
        emit_qk_group(0, 0, 0)
        emit_qk_group(0, 1, 0)
        for ib in range(4):
            emit_v_block(ib)

        pending_proj = []
        reserved_proj = []   # held back as PE filler for the final block

        def pump_proj():
            if pending_proj:
                emit_proj(*pending_proj.pop(0))

        for c in range(NCHUNK):
            for p in range(N_PAIRS):
                fillers = []
                # next attention block's q/k projections
                np_, nc_ = (p + 1, c) if p < N_PAIRS - 1 else (0, c + 1)
                if nc_ < NCHUNK:
                    fillers.append(lambda np_=np_, nc_=nc_: emit_qk_group(np_, 0, nc_))
                    fillers.append(lambda np_=np_, nc_=nc_: emit_qk_group(np_, 1, nc_))
                # V blocks needed soon
                if c == 0:
                    vb0 = 4 * (p + 1)
                    for ib in range(vb0, min(vb0 + 4, 16)):
                        fillers.append(lambda ib=ib: emit_v_block(ib))
                if (p, c) == (N_PAIRS - 1, NCHUNK - 1):
                    fillers += [(lambda t=t: emit_proj(*t))
                                for t in reserved_proj]
                    reserved_proj.clear()
                fillers += [pump_proj] * max(0, (4 * c + 4) - len(fillers))
                emit_attn_pair(p, c, fillers)
                if p == N_PAIRS - 1:
                    tiles = [(ib, ec) for ib in range(4 * c, 4 * c + 4)
                             for ec in range(2)]
                    if c == NCHUNK - 2:
                        pending_proj += tiles[:2]
                        reserved_proj += tiles[2:]
                    else:
                        pending_proj += tiles
        while pending_proj:
            emit_proj(*pending_proj.pop(0))

        if dbg:
            for p in range(4):
                nc.sync.dma_start(out=dbg["qt"].ap()[p], in_=qt_sb[p][:])
                nc.sync.dma_start(out=dbg["kt"].ap()[p], in_=kt_sb[p][:])
                nc.sync.dma_start(out=dbg["attnT"].ap()[p], in_=attnT_sb[p][:])
            for j in range(16):
                nc.sync.dma_start(out=dbg["v"].ap()[j], in_=v_sb[j][:])


_NC_CACHE = None


def _get_nc():
    global _NC_CACHE
    if _NC_CACHE is None:
        _NC_CACHE = _build_program()
    return _NC_CACHE


def _make_in_maps(x, w_qkv, b_qkv, w_out):
    scale = D_HEAD ** -0.5
    in_maps = []
    for core in range(N_CORES):
        b, g = core // 2, core % 2
        f0 = g * F_G
        # xT blob: [p, c*4096 + k*512 + f] = x[b, c*512+f, k*128+p]
        xt = np.ascontiguousarray(x[b].T).astype(np.float16)        # [1024, 2048]
        xt_blob = xt.reshape(KB, 128, NCHUNK, 512).transpose(1, 2, 0, 3) \
                    .reshape(128, NCHUNK * KB * 512)
        # wqk blob: [p, pair*2048 + qk*1024 + kb*128 + f]
        wq = (w_qkv[:, f0:f0 + F_G] * scale).astype(np.float16)      # [1024, 512]
        wk = w_qkv[:, D_MODEL + f0:D_MODEL + f0 + F_G].astype(np.float16)
        wqk_s = np.stack([wq, wk], axis=1)                           # [1024, 2, 512]
        wqk_blob = wqk_s.reshape(KB, 128, 2, N_PAIRS, 128) \
                        .transpose(1, 3, 2, 0, 4).reshape(128, N_PAIRS * 2048)
        # wv blob: [p, kb*512 + f]
        wv_ = w_qkv[:, 2 * D_MODEL + f0:2 * D_MODEL + f0 + F_G].astype(np.float16)
        wv_blob = wv_.reshape(KB, 128, F_G).transpose(1, 0, 2).reshape(128, KB * 512)
        # wout blob: [p, fb*1024 + e] = w_out[f0 + fb*128 + p, e]
        wo = w_out[f0:f0 + F_G, :].astype(np.float16)                # [512, 1024]
        wout_blob = wo.reshape(4, 128, D_MODEL).transpose(1, 0, 2) \
                      .reshape(128, 4 * D_MODEL)
        # bqk: [p, qk*4 + pair]
        bq = (b_qkv[f0:f0 + F_G] * scale).astype(np.float32).reshape(N_PAIRS, 128)
        bk = b_qkv[D_MODEL + f0:D_MODEL + f0 + F_G].astype(np.float32) \
            .reshape(N_PAIRS, 128)
        bqk_blob = np.concatenate([bq, bk], axis=0).T                # [128, 8]
        bv_ = b_qkv[2 * D_MODEL + f0:2 * D_MODEL + f0 + F_G].astype(np.float32)
        in_maps.append({
            "xT": np.ascontiguousarray(xt_blob),
            "wqk": np.ascontiguousarray(wqk_blob),
            "wv": np.ascontiguousarray(wv_blob),
            "wout": np.ascontiguousarray(wout_blob),
            "bqk": np.ascontiguousarray(bqk_blob),
            "bv": np.broadcast_to(bv_, (128, F_G)).copy(),
        })
    return in_maps


def _register_ntff_hook():
    try:
        import antenv.axon_hooks  # noqa: F401
        return
    except ImportError:
        pass
    try:
        from trn_agent_boot.trn_boot import _ntff_profile_via_ctypes
        hook = _ntff_profile_via_ctypes("/opt/axon/libaxon_pjrt.so")
        mod = types.ModuleType("antenv.axon_hooks")
        mod.get_axon_ntff_profile_hook = lambda: hook
        sys.modules["antenv.axon_hooks"] = mod
    except Exception:
        pass


def run(x, w_qkv, b_qkv, w_out, b_out, trace=False, tmpdir=None):
    x = np.asarray(x, dtype=np.float32)
    w_qkv = np.asarray(w_qkv, dtype=np.float32)
    b_qkv = np.asarray(b_qkv, dtype=np.float32)
    w_out = np.asarray(w_out, dtype=np.float32)
    b_out = np.asarray(b_out, dtype=np.float32)

    nc = _get_nc()
    in_maps = _make_in_maps(x, w_qkv, b_qkv, w_out)
    if trace:
        _register_ntff_hook()
    res = run_bass_kernel_spmd(nc, in_maps, core_ids=list(range(N_CORES)),
                               trace=trace, tmpdir=tmpdir)
    bsz = x.shape[0]
    out = np.empty((bsz, N_SEQ, D_MODEL), np.float32)
    for b in range(bsz):
        out[b] = (res.results[2 * b]["y"].astype(np.float32)
                  + res.results[2 * b + 1]["y"].astype(np.float32)
                  + b_out[None, :])
    return out, res


def kernel(x, w_qkv, b_qkv, w_out, b_out):
    out, _ = run(x, w_qkv, b_qkv, w_out, b_out, trace=False)
    return out


# revision 28
# speedup vs baseline: 1.2711x; 1.0390x over previous
"""Multi-head causal self-attention (d_model=1024, 16 heads, seq 2048, batch 4)
as a Bass/Tile kernel for 8 Trainium2 NeuronCores.

Sharding: core c = (batch b = c//2, head-group g = c%2); each group = 8 heads
(512 features), processed as 4 head-PAIRS. Per core:
  - QKV projection for its batch, its group's slice of w_qkv
  - causal attention for its 8 heads (S^T layout, softmax without
    max-subtraction: logits ~ N(0,1), exp is safe in fp16)
  - partial output projection y_part = attn_g @ w_out[g*512:(g+1)*512, :]
Host: y[b] = y_part[2b] + y_part[2b+1] + b_out.

PE-array packing: the two heads of a pair occupy SBUF partitions 0-63 /
64-127, so their K=64 score matmuls run CONCURRENTLY in the top/bottom
row-groups of the 128x128 array (row tiling), and their 64-wide P@V
matmuls run concurrently in the left/right column-groups (col tiling,
tile_position=(0,0)/(0,64)) accumulating into one PSUM bank.  Softmax
denominators are accumulated on DVE from the exp'd tiles (fp16) and
collapsed across partitions with a single ones-vector matmul.

All matmul operands fp16 (PE streams 1 cycle/row vs 4 for fp32);
accumulation fp32 in PSUM.  Host-side input blobs are laid out as the
exact SBUF image ([128, X], >=4KB contiguous per partition row) so input
DMA runs at full HBM rate with ~12 descriptors.
"""
import sys
import types

if "/opt/trn_rl_repo" not in sys.path:
    sys.path.insert(0, "/opt/trn_rl_repo")

import numpy as np

import concourse.bacc as bacc
import concourse.bass as bass
import concourse.mybir as mybir
import concourse.tile as tile
from concourse.bass_utils import run_bass_kernel_spmd
from concourse.masks import make_upper_triangular

D_MODEL = 1024
N_SEQ = 2048
N_HEADS_G = 8          # heads per core (group)
D_HEAD = 64
F_G = N_HEADS_G * D_HEAD   # 512 features per group
N_CORES = 8
N_PAIRS = 4            # head pairs per core
NCHUNK = 4             # 512-wide seq chunks

FP16 = mybir.dt.float16
FP32 = mybir.dt.float32

KB = D_MODEL // 128    # 8 k-blocks


def _build_program():
    nc = bacc.Bacc("TRN2", target_bir_lowering=False, debug=False,
                   num_devices=N_CORES)

    # DRAM blobs are exact SBUF images ([128 partitions, X cols]).
    xT = nc.dram_tensor("xT", [128, NCHUNK * KB * 512], FP16, kind="ExternalInput")
    wqk = nc.dram_tensor("wqk", [128, N_PAIRS * 2048], FP16, kind="ExternalInput")
    wv = nc.dram_tensor("wv", [128, KB * 512], FP16, kind="ExternalInput")
    wout = nc.dram_tensor("wout", [128, 4 * D_MODEL], FP16, kind="ExternalInput")
    bqk = nc.dram_tensor("bqk", [128, 8], FP32, kind="ExternalInput")
    bv = nc.dram_tensor("bv", [128, F_G], FP32, kind="ExternalInput")
    y = nc.dram_tensor("y", [N_SEQ, D_MODEL], FP16, kind="ExternalOutput")
    dbg = {}
    if DEBUG_OUTPUTS:
        dbg["qt"] = nc.dram_tensor("qt_dbg", [4, 128, N_SEQ], FP16,
                                   kind="ExternalOutput")
        dbg["kt"] = nc.dram_tensor("kt_dbg", [4, 128, N_SEQ], FP16,
                                   kind="ExternalOutput")
        dbg["v"] = nc.dram_tensor("v_dbg", [16, 128, N_HEADS_G, D_HEAD + 1],
                                  FP16, kind="ExternalOutput")
        dbg["attnT"] = nc.dram_tensor("attnT_dbg", [4, 128, N_SEQ], FP16,
                                      kind="ExternalOutput")

    with tile.TileContext(nc) as tc:
        _emit(nc, tc, xT, wqk, wv, wout, bqk, bv, y, dbg)
    nc.compile()
    return nc


DEBUG_OUTPUTS = False


def _emit(nc, tc, xT, wqk, wv, wout, bqk, bv, y, dbg=None):
    import contextlib
    ctx = contextlib.ExitStack()
    with ctx:
        persist = ctx.enter_context(tc.tile_pool(name="persist", bufs=1))
        pt_p = ctx.enter_context(tc.tile_pool(name="pt", bufs=6))
        rc_p = ctx.enter_context(tc.tile_pool(name="rc", bufs=3))
        yb_p = ctx.enter_context(tc.tile_pool(name="yb", bufs=2))
        # PSUM: "s" [128,1024]x2 = 4 banks; "acc" [128,512]x2 = 2 banks;
        # "mm" [128,512]x2 = 2 banks (qk/v/proj matmuls share it).
        s_ps = ctx.enter_context(tc.tile_pool(name="sps", bufs=2, space="PSUM"))
        acc_ps = ctx.enter_context(tc.tile_pool(name="accps", bufs=2, space="PSUM"))
        mm_ps = ctx.enter_context(tc.tile_pool(name="mmps", bufs=2, space="PSUM"))

        # ---- persistent SBUF tensors ----
        xT_sb = persist.tile([128, NCHUNK * KB * 512], FP16, tag="xT")
        wqk_sb = persist.tile([128, N_PAIRS * 2048], FP16, tag="wqk")
        wv_sb = persist.tile([128, KB * 512], FP16, tag="wv")
        wout_sb = persist.tile([128, 4 * D_MODEL], FP16, tag="wout")
        bqk_sb = persist.tile([128, 8], FP32, tag="bqk")
        bv_sb = persist.tile([128, F_G], FP32, tag="bv")
        tri_sb = persist.tile([128, 128], FP16, tag="tri")
        ones1_sb = persist.tile([1, 64], FP16, tag="ones1")
        qt_sb = [persist.tile([128, N_SEQ], FP16, tag=f"qt{p}", name=f"qt{p}")
                 for p in range(N_PAIRS)]
        kt_sb = [persist.tile([128, N_SEQ], FP16, tag=f"kt{p}", name=f"kt{p}")
                 for p in range(N_PAIRS)]
        v_sb = [persist.tile([128, N_HEADS_G, D_HEAD + 1], FP16, tag=f"v{j}",
                             name=f"v{j}")
                for j in range(16)]
        attnT_sb = [persist.tile([128, N_SEQ], FP16, tag=f"attnT{p}", name=f"attnT{p}")
                    for p in range(N_PAIRS)]

        # ---- input DMAs, priority order (first matmul needs wqk pair0 + xT c0)
        def load_xt(c, half=None):
            sl = slice(c * 4096, (c + 1) * 4096) if half is None else \
                slice(c * 4096 + half * 2048, c * 4096 + (half + 1) * 2048)
            nc.sync.dma_start(out=xT_sb[:, sl], in_=xT.ap()[:, sl])

        def load_wqk(p):
            nc.sync.dma_start(out=wqk_sb[:, p * 2048:(p + 1) * 2048],
                              in_=wqk.ap()[:, p * 2048:(p + 1) * 2048])

        load_wqk(0)
        load_xt(0, 0)
        load_xt(0, 1)
        nc.sync.dma_start(out=bqk_sb[:], in_=bqk.ap())
        nc.sync.dma_start(out=bv_sb[:], in_=bv.ap())
        nc.sync.dma_start(out=wv_sb[:], in_=wv.ap())
        load_xt(1)
        load_wqk(1)
        nc.sync.dma_start(out=wout_sb[:], in_=wout.ap())
        load_xt(2)
        load_wqk(2)
        load_xt(3)
        load_wqk(3)

        # upper-triangular (incl diag) ones mask: tri[j, i] = 1 iff i >= j
        make_upper_triangular(nc, tri_sb[:], val=1.0, diag=True)
        nc.vector.memset(ones1_sb[:], 1.0)
        # ones column for the fused row-sum in P@V
        for j in range(16):
            nc.vector.memset(v_sb[j][:, :, D_HEAD:D_HEAD + 1], 1.0)

        # ---- QKV projection pieces ----
        def emit_qk_group(p, qk, c):
            # q (qk=0) or k (qk=1) features of head pair p, seq chunk c
            ps = mm_ps.tile([128, 512], FP32, tag="mm", name="qkps")
            for kb in range(KB):
                nc.tensor.matmul(
                    ps[:],
                    wqk_sb[:, p * 2048 + qk * 1024 + kb * 128:
                           p * 2048 + qk * 1024 + (kb + 1) * 128],
                    xT_sb[:, c * 4096 + kb * 512:c * 4096 + (kb + 1) * 512],
                    start=(kb == 0), stop=(kb == KB - 1),
                )
            dest = kt_sb[p] if qk else qt_sb[p]
            nc.vector.tensor_scalar_add(
                dest[:, c * 512:(c + 1) * 512], ps[:],
                bqk_sb[:, qk * 4 + p:qk * 4 + p + 1])

        def emit_v_block(ib):
            c = ib // 4
            ps = mm_ps.tile([128, 512], FP32, tag="mm", name="vps")
            for kb in range(KB):
                nc.tensor.matmul(
                    ps[:],
                    xT_sb[:, c * 4096 + kb * 512 + (ib % 4) * 128:
                          c * 4096 + kb * 512 + (ib % 4 + 1) * 128],
                    wv_sb[:, kb * 512:(kb + 1) * 512],
                    start=(kb == 0), stop=(kb == KB - 1),
                )
            nc.vector.tensor_add(
                v_sb[ib][:, :, 0:D_HEAD],
                ps[:].rearrange("p (h d) -> p h d", h=N_HEADS_G),
                bv_sb[:].rearrange("p (h d) -> p h d", h=N_HEADS_G),
            )

        # ---- causal attention for one head pair, one 512-i chunk ----
        # S^T layout, one PSUM tile [128,1024] per j-block holding the score
        # halves of BOTH heads ([A | B]); the two K=64 score matmuls target
        # array row-groups 0-63 / 64-127 and run concurrently (row tiling).
        # exp is one tight 3D-AP ACTIVATE over both halves.  P@V is the
        # 65-wide fused-sum form (row 64 of the accumulator = sum of exp).
        def emit_attn_pair(p, c, fillers, late_fillers=(), final=False):
            njb = 4 * c + 4
            late_fillers = list(late_fillers)
            acc = [acc_ps.tile([128, 512], FP32, tag="acc", name=f"acch{hl}")
                   for hl in range(2)]
            cols = slice(c * 512, (c + 1) * 512)
            pend_pv = []
            for jb in range(njb):
                if fillers:
                    fillers.pop(0)()
                elif jb >= njb // 2 and late_fillers:
                    late_fillers.pop(0)()
                if jb == njb - 1:
                    while fillers:   # flush unconsumed filler work
                        fillers.pop(0)()
                    while late_fillers:
                        late_fillers.pop(0)()
                t = jb - 4 * c
                off = max(0, t) * 128
                w = 512 - off
                s_t = s_ps.tile([128, 1024], FP32, tag="s", name="sAB")
                for hl in range(2):
                    nc.tensor.matmul(
                        s_t[:, hl * 512 + off:(hl + 1) * 512],
                        kt_sb[p][hl * 64:hl * 64 + 64, jb * 128:(jb + 1) * 128],
                        qt_sb[p][hl * 64:hl * 64 + 64,
                                 c * 512 + off:(c + 1) * 512],
                        start=True, stop=True,
                    )
                pt = pt_p.tile([128, 1024], FP16, tag="pt", name="pt")
                src = bass.AP(tensor=s_t.tensor, offset=s_t.offset + off,
                              ap=[list(s_t.ap[0]), [512, 2], [1, w]])
                dst = bass.AP(tensor=pt.tensor, offset=pt.offset + off,
                              ap=[list(pt.ap[0]), [512, 2], [1, w]])
                nc.scalar.activation(dst, src,
                                     mybir.ActivationFunctionType.Exp)
                if 0 <= t <= 3:
                    blk = bass.AP(tensor=pt.tensor, offset=pt.offset + t * 128,
                                  ap=[list(pt.ap[0]), [512, 2], [1, 128]])
                    tri2 = bass.AP(tensor=tri_sb.tensor, offset=tri_sb.offset,
                                   ap=[list(tri_sb.ap[0]), [0, 2], [1, 128]])
                    nc.vector.tensor_mul(blk, blk, tri2)
                # software-pipeline P@V two j-blocks behind the scores chain
                pend_pv.append((jb, off, pt))
                if len(pend_pv) > 2:
                    emit_pv(p, c, acc, njb, *pend_pv.pop(0))
            while pend_pv:
                emit_pv(p, c, acc, njb, *pend_pv.pop(0))
            # evict accumulators; row 64 = sum(exp)
            ou = [rc_p.tile([D_HEAD + 1, 512], FP16, tag=f"ou{hl}",
                            name=f"ou{hl}") for hl in range(2)]
            for hl in range(2):
                nc.vector.tensor_copy(ou[hl][:], acc[hl][0:D_HEAD + 1, :])
            s48 = rc_p.tile([128, 8], FP16, tag="s48", name="s48")
            for hl in range(2):
                nc.sync.dma_start(out=s48[:, 4 * hl:4 * hl + 4],
                                  in_=ou[hl][D_HEAD:D_HEAD + 1, :])
            r48 = rc_p.tile([128, 8], FP16, tag="r48", name="r48")
            with nc.allow_low_precision(reason="1/softmax-denom in fp16 "
                                        "is ~0.05% rel err"):
                nc.vector.reciprocal(r48[:], s48[:])
            rr = rc_p.tile([1, 1024], FP16, tag="rr", name="rr")
            # per-head halves: [1,512] <- [128,4] keeps i = 4p+c identity
            nc.sync.dma_start(out=rr[:, 0:512], in_=r48[:, 0:4])
            nc.sync.dma_start(out=rr[:, 512:1024], in_=r48[:, 4:8])
            if not final:
                # gpsimd broadcast (latency hidden under the next block)
                rep = rc_p.tile([128, 1024], FP16, tag="rep", name="rep")
                nc.gpsimd.partition_broadcast(rep[:], rr[:])
                repA, repB = rep[0:64, 0:512], rep[0:64, 512:1024]
            else:
                # latency-critical last block: PE ones-matmul broadcast
                reps = []
                for hl in range(2):
                    rp = mm_ps.tile([64, 512], FP32, tag="mm", name="repps")
                    nc.tensor.matmul(rp[:], ones1_sb[:],
                                     rr[:, hl * 512:(hl + 1) * 512],
                                     start=True, stop=True)
                    reps.append(rp)
                repA, repB = reps[0][:], reps[1][:]
            nc.vector.tensor_mul(attnT_sb[p][0:64, cols], ou[0][0:D_HEAD, :],
                                 repA)
            tmp = rc_p.tile([64, 512], FP16, tag="tmpB", name="tmpB")
            nc.vector.tensor_mul(tmp[:], ou[1][0:D_HEAD, :], repB)
            nc.sync.dma_start(out=attnT_sb[p][64:128, cols], in_=tmp[:])

        def emit_pv(p, c, acc, njb, jb, off, pt):
            for hl in range(2):
                nc.tensor.matmul(
                    acc[hl][0:D_HEAD + 1, off:512],
                    v_sb[jb][:, 2 * p + hl, :],
                    pt[:, hl * 512 + off:(hl + 1) * 512],
                    start=(jb == 0), stop=(jb == njb - 1),
                )

        # ---- output projection ----
        y_sb = {}

        def emit_proj(ib, ec, tail=False):
            if tail:   # post-attention: the score pool's banks are free
                ps = s_ps.tile([128, 512], FP32, tag="s", name="projps")
            else:
                ps = mm_ps.tile([128, 512], FP32, tag="mm", name="projps")
            for fbp in range(N_PAIRS):
                nc.tensor.matmul(
                    ps[:],
                    attnT_sb[fbp][:, ib * 128:(ib + 1) * 128],
                    wout_sb[:, fbp * 1024 + ec * 512:fbp * 1024 + (ec + 1) * 512],
                    start=(fbp == 0), stop=(fbp == N_PAIRS - 1),
                )
            if ib not in y_sb:
                y_sb[ib] = yb_p.tile([128, D_MODEL], FP16, tag="ysb", name="ysb")
            nc.vector.tensor_copy(y_sb[ib][:, ec * 512:(ec + 1) * 512], ps[:])
            if ec == 1:
                nc.sync.dma_start(
                    out=y.ap()[ib * 128:(ib + 1) * 128, :], in_=y_sb.pop(ib)[:])

        # ---- schedule ----
        emit_qk_group(0, 0, 0)
        emit_qk_group(0, 1, 0)
        for ib in range(4):
            emit_v_block(ib)

        pending_proj = []
        reserved_proj = []   # held back as PE filler for the final block

        def pump_proj():
            if pending_proj:
                emit_proj(*pending_proj.pop(0))

        for c in range(NCHUNK):
            for p in range(N_PAIRS):
                final = (p, c) == (N_PAIRS - 1, NCHUNK - 1)
                fillers = []
                # next attention block's q/k projections
                np_, nc_ = (p + 1, c) if p < N_PAIRS - 1 else (0, c + 1)
                if nc_ < NCHUNK:
                    fillers.append(lambda np_=np_, nc_=nc_: emit_qk_group(np_, 0, nc_))
                    fillers.append(lambda np_=np_, nc_=nc_: emit_qk_group(np_, 1, nc_))
                # V blocks needed soon
                if c == 0:
                    vb0 = 4 * (p + 1)
                    for ib in range(vb0, min(vb0 + 4, 16)):
                        fillers.append(lambda ib=ib: emit_v_block(ib))
                # output projections go in the block's second half so their
                # matmul-count waits don't gate this block's exps on the
                # previous block's normalize chain
                late = []
                if final:
                    late += [(lambda t=t: emit_proj(*t)) for t in reserved_proj]
                    reserved_proj.clear()
                late += [pump_proj] * max(0, (4 * c + 4) // 2 - len(late))
                emit_attn_pair(p, c, fillers, late, final=final)
                if p == N_PAIRS - 1:
                    tiles = [(ib, ec) for ib in range(4 * c, 4 * c + 4)
                             for ec in range(2)]
                    if c == NCHUNK - 2:
                        pending_proj += tiles[:2]
                        reserved_proj += tiles[2:]
                    else:
                        pending_proj += tiles
        for i, t in enumerate(pending_proj):
            emit_proj(*t, tail=(i % 2 == 0))
        pending_proj.clear()

        if dbg:
            for p in range(4):
                nc.sync.dma_start(out=dbg["qt"].ap()[p], in_=qt_sb[p][:])
                nc.sync.dma_start(out=dbg["kt"].ap()[p], in_=kt_sb[p][:])
                nc.sync.dma_start(out=dbg["attnT"].ap()[p], in_=attnT_sb[p][:])
            for j in range(16):
                nc.sync.dma_start(out=dbg["v"].ap()[j], in_=v_sb[j][:])


_NC_CACHE = None


def _get_nc():
    global _NC_CACHE
    if _NC_CACHE is None:
        _NC_CACHE = _build_program()
    return _NC_CACHE


def _make_in_maps(x, w_qkv, b_qkv, w_out):
    scale = D_HEAD ** -0.5
    in_maps = []
    for core in range(N_CORES):
        b, g = core // 2, core % 2
        f0 = g * F_G
        # xT blob: [p, c*4096 + k*512 + f] = x[b, c*512+f, k*128+p]
        xt = np.ascontiguousarray(x[b].T).astype(np.float16)        # [1024, 2048]
        xt_blob = xt.reshape(KB, 128, NCHUNK, 512).transpose(1, 2, 0, 3) \
                    .reshape(128, NCHUNK * KB * 512)
        # wqk blob: [p, pair*2048 + qk*1024 + kb*128 + f]
        wq = (w_qkv[:, f0:f0 + F_G] * scale).astype(np.float16)      # [1024, 512]
        wk = w_qkv[:, D_MODEL + f0:D_MODEL + f0 + F_G].astype(np.float16)
        wqk_s = np.stack([wq, wk], axis=1)                           # [1024, 2, 512]
        wqk_blob = wqk_s.reshape(KB, 128, 2, N_PAIRS, 128) \
                        .transpose(1, 3, 2, 0, 4).reshape(128, N_PAIRS * 2048)
        # wv blob: [p, kb*512 + f]
        wv_ = w_qkv[:, 2 * D_MODEL + f0:2 * D_MODEL + f0 + F_G].astype(np.float16)
        wv_blob = wv_.reshape(KB, 128, F_G).transpose(1, 0, 2).reshape(128, KB * 512)
        # wout blob: [p, fb*1024 + e] = w_out[f0 + fb*128 + p, e]
        wo = w_out[f0:f0 + F_G, :].astype(np.float16)                # [512, 1024]
        wout_blob = wo.reshape(4, 128, D_MODEL).transpose(1, 0, 2) \
                      .reshape(128, 4 * D_MODEL)
        # bqk: [p, qk*4 + pair]
        bq = (b_qkv[f0:f0 + F_G] * scale).astype(np.float32).reshape(N_PAIRS, 128)
        bk = b_qkv[D_MODEL + f0:D_MODEL + f0 + F_G].astype(np.float32) \
            .reshape(N_PAIRS, 128)
        bqk_blob = np.concatenate([bq, bk], axis=0).T                # [128, 8]
        bv_ = b_qkv[2 * D_MODEL + f0:2 * D_MODEL + f0 + F_G].astype(np.float32)
        in_maps.append({
            "xT": np.ascontiguousarray(xt_blob),
            "wqk": np.ascontiguousarray(wqk_blob),
            "wv": np.ascontiguousarray(wv_blob),
            "wout": np.ascontiguousarray(wout_blob),
            "bqk": np.ascontiguousarray(bqk_blob),
            "bv": np.broadcast_to(bv_, (128, F_G)).copy(),
        })
    return in_maps


def _register_ntff_hook():
    try:
        import antenv.axon_hooks  # noqa: F401
        return
    except ImportError:
        pass
    try:
        from trn_agent_boot.trn_boot import _ntff_profile_via_ctypes
        hook = _ntff_profile_via_ctypes("/opt/axon/libaxon_pjrt.so")
        mod = types.ModuleType("antenv.axon_hooks")
        mod.get_axon_ntff_profile_hook = lambda: hook
        sys.modules["antenv.axon_hooks"] = mod
    except Exception:
        pass


def run(x, w_qkv, b_qkv, w_out, b_out, trace=False, tmpdir=None):
    x = np.asarray(x, dtype=np.float32)
    w_qkv = np.asarray(w_qkv, dtype=np.float32)
    b_qkv = np.asarray(b_qkv, dtype=np.float32)
    w_out = np.asarray(w_out, dtype=np.float32)
    b_out = np.asarray(b_out, dtype=np.float32)

    nc = _get_nc()
    in_maps = _make_in_maps(x, w_qkv, b_qkv, w_out)
    if trace:
        _register_ntff_hook()
    res = run_bass_kernel_spmd(nc, in_maps, core_ids=list(range(N_CORES)),
                               trace=trace, tmpdir=tmpdir)
    bsz = x.shape[0]
    out = np.empty((bsz, N_SEQ, D_MODEL), np.float32)
    for b in range(bsz):
        out[b] = (res.results[2 * b]["y"].astype(np.float32)
                  + res.results[2 * b + 1]["y"].astype(np.float32)
                  + b_out[None, :])
    return out, res


def kernel(x, w_qkv, b_qkv, w_out, b_out):
    out, _ = run(x, w_qkv, b_qkv, w_out, b_out, trace=False)
    return out


# revision 30
# speedup vs baseline: 1.2913x; 1.0159x over previous
"""Multi-head causal self-attention (d_model=1024, 16 heads, seq 2048, batch 4)
as a Bass/Tile kernel for 8 Trainium2 NeuronCores.

Sharding: core c = (batch b = c//2, head-group g = c%2); each group = 8 heads
(512 features), processed as 4 head-PAIRS. Per core:
  - QKV projection for its batch, its group's slice of w_qkv
  - causal attention for its 8 heads (S^T layout, softmax without
    max-subtraction: logits ~ N(0,1), exp is safe in fp16)
  - partial output projection y_part = attn_g @ w_out[g*512:(g+1)*512, :]
Host: y[b] = y_part[2b] + y_part[2b+1] + b_out.

PE-array packing: the two heads of a pair occupy SBUF partitions 0-63 /
64-127, so their K=64 score matmuls run CONCURRENTLY in the top/bottom
row-groups of the 128x128 array (row tiling), and their 64-wide P@V
matmuls run concurrently in the left/right column-groups (col tiling,
tile_position=(0,0)/(0,64)) accumulating into one PSUM bank.  Softmax
denominators are accumulated on DVE from the exp'd tiles (fp16) and
collapsed across partitions with a single ones-vector matmul.

All matmul operands fp16 (PE streams 1 cycle/row vs 4 for fp32);
accumulation fp32 in PSUM.  Host-side input blobs are laid out as the
exact SBUF image ([128, X], >=4KB contiguous per partition row) so input
DMA runs at full HBM rate with ~12 descriptors.
"""
import sys
import types

if "/opt/trn_rl_repo" not in sys.path:
    sys.path.insert(0, "/opt/trn_rl_repo")

import numpy as np

import concourse.bacc as bacc
import concourse.bass as bass
import concourse.mybir as mybir
import concourse.tile as tile
from concourse.bass_utils import run_bass_kernel_spmd
from concourse.masks import make_upper_triangular

D_MODEL = 1024
N_SEQ = 2048
N_HEADS_G = 8          # heads per core (group)
D_HEAD = 64
F_G = N_HEADS_G * D_HEAD   # 512 features per group
N_CORES = 8
N_PAIRS = 4            # head pairs per core
NCHUNK = 4             # 512-wide seq chunks

FP16 = mybir.dt.float16
FP32 = mybir.dt.float32

KB = D_MODEL // 128    # 8 k-blocks


def _build_program():
    nc = bacc.Bacc("TRN2", target_bir_lowering=False, debug=False,
                   num_devices=N_CORES)

    # DRAM blobs are exact SBUF images ([128 partitions, X cols]).
    xT = nc.dram_tensor("xT", [128, NCHUNK * KB * 512], FP16, kind="ExternalInput")
    wqk = nc.dram_tensor("wqk", [128, N_PAIRS * 2048], FP16, kind="ExternalInput")
    wv = nc.dram_tensor("wv", [128, KB * 512], FP16, kind="ExternalInput")
    wout = nc.dram_tensor("wout", [128, 4 * D_MODEL], FP16, kind="ExternalInput")
    bqk = nc.dram_tensor("bqk", [128, 8], FP32, kind="ExternalInput")
    bv = nc.dram_tensor("bv", [128, F_G], FP32, kind="ExternalInput")
    y = nc.dram_tensor("y", [N_SEQ, D_MODEL], FP16, kind="ExternalOutput")
    dbg = {}
    if DEBUG_OUTPUTS:
        dbg["qt"] = nc.dram_tensor("qt_dbg", [4, 128, N_SEQ], FP16,
                                   kind="ExternalOutput")
        dbg["kt"] = nc.dram_tensor("kt_dbg", [4, 128, N_SEQ], FP16,
                                   kind="ExternalOutput")
        dbg["v"] = nc.dram_tensor("v_dbg", [16, 128, N_HEADS_G, D_HEAD + 1],
                                  FP16, kind="ExternalOutput")
        dbg["attnT"] = nc.dram_tensor("attnT_dbg", [4, 128, N_SEQ], FP16,
                                      kind="ExternalOutput")

    with tile.TileContext(nc) as tc:
        _emit(nc, tc, xT, wqk, wv, wout, bqk, bv, y, dbg)
    nc.compile()
    return nc


DEBUG_OUTPUTS = False


def _emit(nc, tc, xT, wqk, wv, wout, bqk, bv, y, dbg=None):
    import contextlib
    ctx = contextlib.ExitStack()
    with ctx:
        persist = ctx.enter_context(tc.tile_pool(name="persist", bufs=1))
        pt_p = ctx.enter_context(tc.tile_pool(name="pt", bufs=6))
        rc_p = ctx.enter_context(tc.tile_pool(name="rc", bufs=3))
        yb_p = ctx.enter_context(tc.tile_pool(name="yb", bufs=2))
        # PSUM: "s" [128,1024]x2 = 4 banks; "acc" [128,512]x2 = 2 banks;
        # "mm" [128,512]x2 = 2 banks (qk/v/proj matmuls share it).
        s_ps = ctx.enter_context(tc.tile_pool(name="sps", bufs=2, space="PSUM"))
        acc_ps = ctx.enter_context(tc.tile_pool(name="accps", bufs=2, space="PSUM"))
        mm_ps = ctx.enter_context(tc.tile_pool(name="mmps", bufs=2, space="PSUM"))

        # ---- persistent SBUF tensors ----
        xT_sb = persist.tile([128, NCHUNK * KB * 512], FP16, tag="xT")
        wqk_sb = persist.tile([128, N_PAIRS * 2048], FP16, tag="wqk")
        wv_sb = persist.tile([128, KB * 512], FP16, tag="wv")
        wout_sb = persist.tile([128, 4 * D_MODEL], FP16, tag="wout")
        bqk_sb = persist.tile([128, 8], FP32, tag="bqk")
        bv_sb = persist.tile([128, F_G], FP32, tag="bv")
        tri_sb = persist.tile([128, 128], FP16, tag="tri")
        ones1_sb = persist.tile([1, 64], FP16, tag="ones1")
        qt_sb = [persist.tile([128, N_SEQ], FP16, tag=f"qt{p}", name=f"qt{p}")
                 for p in range(N_PAIRS)]
        kt_sb = [persist.tile([128, N_SEQ], FP16, tag=f"kt{p}", name=f"kt{p}")
                 for p in range(N_PAIRS)]
        v_sb = [persist.tile([128, N_HEADS_G, D_HEAD + 1], FP16, tag=f"v{j}",
                             name=f"v{j}")
                for j in range(16)]
        attnT_sb = [persist.tile([128, N_SEQ], FP16, tag=f"attnT{p}", name=f"attnT{p}")
                    for p in range(N_PAIRS)]

        # ---- input DMAs, priority order (first matmul needs wqk pair0 + xT c0)
        def load_xt(c, half=None):
            sl = slice(c * 4096, (c + 1) * 4096) if half is None else \
                slice(c * 4096 + half * 2048, c * 4096 + (half + 1) * 2048)
            nc.sync.dma_start(out=xT_sb[:, sl], in_=xT.ap()[:, sl])

        def load_wqk(p):
            nc.sync.dma_start(out=wqk_sb[:, p * 2048:(p + 1) * 2048],
                              in_=wqk.ap()[:, p * 2048:(p + 1) * 2048])

        load_wqk(0)
        load_xt(0, 0)
        load_xt(0, 1)
        nc.sync.dma_start(out=bqk_sb[:], in_=bqk.ap())
        nc.sync.dma_start(out=bv_sb[:], in_=bv.ap())
        nc.sync.dma_start(out=wv_sb[:], in_=wv.ap())
        load_xt(1)
        load_wqk(1)
        nc.sync.dma_start(out=wout_sb[:], in_=wout.ap())
        load_xt(2)
        load_wqk(2)
        load_xt(3)
        load_wqk(3)

        # upper-triangular (incl diag) ones mask: tri[j, i] = 1 iff i >= j
        make_upper_triangular(nc, tri_sb[:], val=1.0, diag=True)
        nc.vector.memset(ones1_sb[:], 1.0)
        # ones column for the fused row-sum in P@V
        for j in range(16):
            nc.vector.memset(v_sb[j][:, :, D_HEAD:D_HEAD + 1], 1.0)

        # ---- QKV projection pieces ----
        def emit_qk_group(p, qk, c):
            # q (qk=0) or k (qk=1) features of head pair p, seq chunk c
            ps = mm_ps.tile([128, 512], FP32, tag="mm", name="qkps")
            for kb in range(KB):
                nc.tensor.matmul(
                    ps[:],
                    wqk_sb[:, p * 2048 + qk * 1024 + kb * 128:
                           p * 2048 + qk * 1024 + (kb + 1) * 128],
                    xT_sb[:, c * 4096 + kb * 512:c * 4096 + (kb + 1) * 512],
                    start=(kb == 0), stop=(kb == KB - 1),
                )
            dest = kt_sb[p] if qk else qt_sb[p]
            nc.vector.tensor_scalar_add(
                dest[:, c * 512:(c + 1) * 512], ps[:],
                bqk_sb[:, qk * 4 + p:qk * 4 + p + 1])

        def emit_v_block(ib):
            c = ib // 4
            ps = mm_ps.tile([128, 512], FP32, tag="mm", name="vps")
            for kb in range(KB):
                nc.tensor.matmul(
                    ps[:],
                    xT_sb[:, c * 4096 + kb * 512 + (ib % 4) * 128:
                          c * 4096 + kb * 512 + (ib % 4 + 1) * 128],
                    wv_sb[:, kb * 512:(kb + 1) * 512],
                    start=(kb == 0), stop=(kb == KB - 1),
                )
            nc.vector.tensor_add(
                v_sb[ib][:, :, 0:D_HEAD],
                ps[:].rearrange("p (h d) -> p h d", h=N_HEADS_G),
                bv_sb[:].rearrange("p (h d) -> p h d", h=N_HEADS_G),
            )

        # ---- causal attention for one head pair, one 512-i chunk ----
        # S^T layout, one PSUM tile [128,1024] per j-block holding the score
        # halves of BOTH heads ([A | B]); the two K=64 score matmuls target
        # array row-groups 0-63 / 64-127 and run concurrently (row tiling).
        # exp is one tight 3D-AP ACTIVATE over both halves.  P@V is the
        # 65-wide fused-sum form (row 64 of the accumulator = sum of exp).
        def emit_attn_pair(p, c, fillers, late_fillers=(), final=False):
            njb = 4 * c + 4
            late_fillers = list(late_fillers)
            acc = [acc_ps.tile([128, 512], FP32, tag="acc", name=f"acch{hl}")
                   for hl in range(2)]
            cols = slice(c * 512, (c + 1) * 512)
            pend_pv = []
            for jb in range(njb):
                if fillers:
                    fillers.pop(0)()
                elif jb >= njb // 2 and late_fillers:
                    late_fillers.pop(0)()
                if jb == njb - 1:
                    while fillers:   # flush unconsumed filler work
                        fillers.pop(0)()
                    while late_fillers:
                        late_fillers.pop(0)()
                t = jb - 4 * c
                off = max(0, t) * 128
                w = 512 - off
                s_t = s_ps.tile([128, 1024], FP32, tag="s", name="sAB")
                for hl in range(2):
                    nc.tensor.matmul(
                        s_t[:, hl * 512 + off:(hl + 1) * 512],
                        kt_sb[p][hl * 64:hl * 64 + 64, jb * 128:(jb + 1) * 128],
                        qt_sb[p][hl * 64:hl * 64 + 64,
                                 c * 512 + off:(c + 1) * 512],
                        start=True, stop=True,
                    )
                pt = pt_p.tile([128, 1024], FP16, tag="pt", name="pt")
                src = bass.AP(tensor=s_t.tensor, offset=s_t.offset + off,
                              ap=[list(s_t.ap[0]), [512, 2], [1, w]])
                dst = bass.AP(tensor=pt.tensor, offset=pt.offset + off,
                              ap=[list(pt.ap[0]), [512, 2], [1, w]])
                nc.scalar.activation(dst, src,
                                     mybir.ActivationFunctionType.Exp)
                if 0 <= t <= 3:
                    blk = bass.AP(tensor=pt.tensor, offset=pt.offset + t * 128,
                                  ap=[list(pt.ap[0]), [512, 2], [1, 128]])
                    tri2 = bass.AP(tensor=tri_sb.tensor, offset=tri_sb.offset,
                                   ap=[list(tri_sb.ap[0]), [0, 2], [1, 128]])
                    nc.vector.tensor_mul(blk, blk, tri2)
                # software-pipeline P@V two j-blocks behind the scores chain
                pend_pv.append((jb, off, pt))
                if len(pend_pv) > 2:
                    emit_pv(p, c, acc, njb, *pend_pv.pop(0))
            while pend_pv:
                emit_pv(p, c, acc, njb, *pend_pv.pop(0))
            # evict accumulators; row 64 = sum(exp)
            ou = [rc_p.tile([D_HEAD + 1, 512], FP16, tag=f"ou{hl}",
                            name=f"ou{hl}") for hl in range(2)]
            for hl in range(2):
                nc.vector.tensor_copy(ou[hl][:], acc[hl][0:D_HEAD + 1, :])
            s48 = rc_p.tile([128, 8], FP16, tag="s48", name="s48")
            for hl in range(2):
                nc.sync.dma_start(out=s48[:, 4 * hl:4 * hl + 4],
                                  in_=ou[hl][D_HEAD:D_HEAD + 1, :])
            state = {"ou": ou, "s48": s48}

            # The rest of the normalize chain is deferred: its DVE ops wait
            # on DMA/gpsimd latency, and at the DVE queue head they would
            # stall the next block's masks (count-based semaphores).
            def chain_a():
                r48 = rc_p.tile([128, 8], FP16, tag="r48", name="r48")
                with nc.allow_low_precision(reason="1/softmax-denom in fp16 "
                                            "is ~0.05% rel err"):
                    nc.vector.reciprocal(r48[:], state["s48"][:])
                rr = rc_p.tile([1, 1024], FP16, tag="rr", name="rr")
                # per-head halves: [1,512] <- [128,4] keeps i=4p+c identity
                nc.sync.dma_start(out=rr[:, 0:512], in_=r48[:, 0:4])
                nc.sync.dma_start(out=rr[:, 512:1024], in_=r48[:, 4:8])
                if not final:
                    rep = rc_p.tile([128, 1024], FP16, tag="rep", name="rep")
                    nc.gpsimd.partition_broadcast(rep[:], rr[:])
                    state["repA"] = rep[0:64, 0:512]
                    state["repB"] = rep[0:64, 512:1024]
                else:
                    # latency-critical last block: PE ones-matmul broadcast
                    reps = []
                    for hl in range(2):
                        rp = mm_ps.tile([64, 512], FP32, tag="mm", name="repps")
                        nc.tensor.matmul(rp[:], ones1_sb[:],
                                         rr[:, hl * 512:(hl + 1) * 512],
                                         start=True, stop=True)
                        reps.append(rp)
                    state["repA"], state["repB"] = reps[0][:], reps[1][:]

            def chain_b():
                ou = state["ou"]
                nc.vector.tensor_mul(attnT_sb[p][0:64, cols],
                                     ou[0][0:D_HEAD, :], state["repA"])
                tmp = rc_p.tile([64, 512], FP16, tag="tmpB", name="tmpB")
                nc.vector.tensor_mul(tmp[:], ou[1][0:D_HEAD, :], state["repB"])
                nc.sync.dma_start(out=attnT_sb[p][64:128, cols], in_=tmp[:])

            return chain_a, chain_b

        def emit_pv(p, c, acc, njb, jb, off, pt):
            for hl in range(2):
                nc.tensor.matmul(
                    acc[hl][0:D_HEAD + 1, off:512],
                    v_sb[jb][:, 2 * p + hl, :],
                    pt[:, hl * 512 + off:(hl + 1) * 512],
                    start=(jb == 0), stop=(jb == njb - 1),
                )

        # ---- output projection ----
        y_sb = {}

        def emit_proj(ib, ec, tail=False):
            if tail:   # post-attention: the score pool's banks are free
                ps = s_ps.tile([128, 512], FP32, tag="s", name="projps")
            else:
                ps = mm_ps.tile([128, 512], FP32, tag="mm", name="projps")
            for fbp in range(N_PAIRS):
                nc.tensor.matmul(
                    ps[:],
                    attnT_sb[fbp][:, ib * 128:(ib + 1) * 128],
                    wout_sb[:, fbp * 1024 + ec * 512:fbp * 1024 + (ec + 1) * 512],
                    start=(fbp == 0), stop=(fbp == N_PAIRS - 1),
                )
            if ib not in y_sb:
                y_sb[ib] = yb_p.tile([128, D_MODEL], FP16, tag="ysb", name="ysb")
            nc.vector.tensor_copy(y_sb[ib][:, ec * 512:(ec + 1) * 512], ps[:])
            if ec == 1:
                nc.sync.dma_start(
                    out=y.ap()[ib * 128:(ib + 1) * 128, :], in_=y_sb.pop(ib)[:])

        # ---- schedule ----
        emit_qk_group(0, 0, 0)
        emit_qk_group(0, 1, 0)
        for ib in range(4):
            emit_v_block(ib)

        pending_proj = []
        reserved_proj = []   # held back as PE filler for the final block

        def pump_proj():
            if pending_proj:
                emit_proj(*pending_proj.pop(0))

        chains = None   # deferred normalize chain of the previous block
        for c in range(NCHUNK):
            for p in range(N_PAIRS):
                final = (p, c) == (N_PAIRS - 1, NCHUNK - 1)
                fillers = []
                # next attention block's q/k projections
                np_, nc_ = (p + 1, c) if p < N_PAIRS - 1 else (0, c + 1)
                if nc_ < NCHUNK:
                    fillers.append(lambda np_=np_, nc_=nc_: emit_qk_group(np_, 0, nc_))
                    fillers.append(lambda np_=np_, nc_=nc_: emit_qk_group(np_, 1, nc_))
                # previous block's deferred chain: recip at jb1, muls at jb>=3
                # (keeps their DMA/gpsimd-latency waits off the DVE queue head)
                if chains:
                    fillers.insert(1, chains[0])
                    fillers.insert(3, chains[1])
                # V blocks needed soon
                if c == 0:
                    vb0 = 4 * (p + 1)
                    for ib in range(vb0, min(vb0 + 4, 16)):
                        fillers.append(lambda ib=ib: emit_v_block(ib))
                # output projections go in the block's second half so their
                # matmul-count waits don't gate this block's exps on the
                # previous block's normalize chain
                late = []
                if final:
                    late += [(lambda t=t: emit_proj(*t)) for t in reserved_proj]
                    reserved_proj.clear()
                late += [pump_proj] * max(0, (4 * c + 4) // 2 - len(late))
                chains = emit_attn_pair(p, c, fillers, late, final=final)
                if p == N_PAIRS - 1:
                    tiles = [(ib, ec) for ib in range(4 * c, 4 * c + 4)
                             for ec in range(2)]
                    if c == NCHUNK - 2:
                        pending_proj += tiles[:2]
                        reserved_proj += tiles[2:]
                    else:
                        pending_proj += tiles
        # final block's chain runs inline (PE-broadcast variant)
        chains[0]()
        chains[1]()
        for i, t in enumerate(pending_proj):
            emit_proj(*t, tail=(i % 2 == 0))
        pending_proj.clear()

        if dbg:
            for p in range(4):
                nc.sync.dma_start(out=dbg["qt"].ap()[p], in_=qt_sb[p][:])
                nc.sync.dma_start(out=dbg["kt"].ap()[p], in_=kt_sb[p][:])
                nc.sync.dma_start(out=dbg["attnT"].ap()[p], in_=attnT_sb[p][:])
            for j in range(16):
                nc.sync.dma_start(out=dbg["v"].ap()[j], in_=v_sb[j][:])


_NC_CACHE = None


def _get_nc():
    global _NC_CACHE
    if _NC_CACHE is None:
        _NC_CACHE = _build_program()
    return _NC_CACHE


def _make_in_maps(x, w_qkv, b_qkv, w_out):
    scale = D_HEAD ** -0.5
    in_maps = []
    for core in range(N_CORES):
        b, g = core // 2, core % 2
        f0 = g * F_G
        # xT blob: [p, c*4096 + k*512 + f] = x[b, c*512+f, k*128+p]
        xt = np.ascontiguousarray(x[b].T).astype(np.float16)        # [1024, 2048]
        xt_blob = xt.reshape(KB, 128, NCHUNK, 512).transpose(1, 2, 0, 3) \
                    .reshape(128, NCHUNK * KB * 512)
        # wqk blob: [p, pair*2048 + qk*1024 + kb*128 + f]
        wq = (w_qkv[:, f0:f0 + F_G] * scale).astype(np.float16)      # [1024, 512]
        wk = w_qkv[:, D_MODEL + f0:D_MODEL + f0 + F_G].astype(np.float16)
        wqk_s = np.stack([wq, wk], axis=1)                           # [1024, 2, 512]
        wqk_blob = wqk_s.reshape(KB, 128, 2, N_PAIRS, 128) \
                        .transpose(1, 3, 2, 0, 4).reshape(128, N_PAIRS * 2048)
        # wv blob: [p, kb*512 + f]
        wv_ = w_qkv[:, 2 * D_MODEL + f0:2 * D_MODEL + f0 + F_G].astype(np.float16)
        wv_blob = wv_.reshape(KB, 128, F_G).transpose(1, 0, 2).reshape(128, KB * 512)
        # wout blob: [p, fb*1024 + e] = w_out[f0 + fb*128 + p, e]
        wo = w_out[f0:f0 + F_G, :].astype(np.float16)                # [512, 1024]
        wout_blob = wo.reshape(4, 128, D_MODEL).transpose(1, 0, 2) \
                      .reshape(128, 4 * D_MODEL)
        # bqk: [p, qk*4 + pair]
        bq = (b_qkv[f0:f0 + F_G] * scale).astype(np.float32).reshape(N_PAIRS, 128)
        bk = b_qkv[D_MODEL + f0:D_MODEL + f0 + F_G].astype(np.float32) \
            .reshape(N_PAIRS, 128)
        bqk_blob = np.concatenate([bq, bk], axis=0).T                # [128, 8]
        bv_ = b_qkv[2 * D_MODEL + f0:2 * D_MODEL + f0 + F_G].astype(np.float32)
        in_maps.append({
            "xT": np.ascontiguousarray(xt_blob),
            "wqk": np.ascontiguousarray(wqk_blob),
            "wv": np.ascontiguousarray(wv_blob),
            "wout": np.ascontiguousarray(wout_blob),
            "bqk": np.ascontiguousarray(bqk_blob),
            "bv": np.broadcast_to(bv_, (128, F_G)).copy(),
        })
    return in_maps


def _register_ntff_hook():
    try:
        import antenv.axon_hooks  # noqa: F401
        return
    except ImportError:
        pass
    try:
        from trn_agent_boot.trn_boot import _ntff_profile_via_ctypes
        hook = _ntff_profile_via_ctypes("/opt/axon/libaxon_pjrt.so")
        mod = types.ModuleType("antenv.axon_hooks")
        mod.get_axon_ntff_profile_hook = lambda: hook
        sys.modules["antenv.axon_hooks"] = mod
    except Exception:
        pass


def run(x, w_qkv, b_qkv, w_out, b_out, trace=False, tmpdir=None):
    x = np.asarray(x, dtype=np.float32)
    w_qkv = np.asarray(w_qkv, dtype=np.float32)
    b_qkv = np.asarray(b_qkv, dtype=np.float32)
    w_out = np.asarray(w_out, dtype=np.float32)
    b_out = np.asarray(b_out, dtype=np.float32)

    nc = _get_nc()
    in_maps = _make_in_maps(x, w_qkv, b_qkv, w_out)
    if trace:
        _register_ntff_hook()
    res = run_bass_kernel_spmd(nc, in_maps, core_ids=list(range(N_CORES)),
                               trace=trace, tmpdir=tmpdir)
    bsz = x.shape[0]
    out = np.empty((bsz, N_SEQ, D_MODEL), np.float32)
    for b in range(bsz):
        out[b] = (res.results[2 * b]["y"].astype(np.float32)
                  + res.results[2 * b + 1]["y"].astype(np.float32)
                  + b_out[None, :])
    return out, res


def kernel(x, w_qkv, b_qkv, w_out, b_out):
    out, _ = run(x, w_qkv, b_qkv, w_out, b_out, trace=False)
    return out
